# revision 4
# baseline (speedup 1.0000x reference)
"""Trainium2 Bass kernel for AtomTypeGNN message passing.

Computation (reference):
    adj_exp[m, f] = sum_n dist_adj[m, n] * dist_exp[m, n, f]          # [N, F]
    feat[m, k]    = sum_{f,h} adj_exp[m, f] * W[f, h, k] * emb[m, h]  # [N, K]
    out           = softplus(feat) + b                                # [N, K]

Sharding: rows m across 8 cores (256 rows each); W/b replicated. No
cross-core communication needed.

All large inputs are cast to fp16 AND pre-transposed on the host into
partition-major contiguous layouts, so every streaming DMA moves 16KB
of contiguous DRAM per partition (line-rate HBM).  Accumulation stays
fp32 in PSUM.

Per-core device algorithm (m-blocks of 128 rows):
  Step 1 on the TensorEngine: for each m and each 128-wide n-chunk j,
    psum_adj[32r, q0*64+f] += A_col(m,j)[128n,1].T @ E_chunk(m,j)[128n,64f]
  with m_loc = r*32 + q0; the four r groups hold 1-col A weights at PE
  column-strips 32r (tile_position) so LDWEIGHTS overlaps matmuls.
  E arrives as e_prep[p, mb, q0, r, j*64+f] (partition p holds
  n in [16p,16p+16)); one DMA fetches a q0-pair = [128, 2, 4, 1024].
  Step 2: psum -> scratch (DVE+GpSimd halves), SWDGE redistribute to
  adj_exp[m, f], then per f: tensor_scalar (DVE) + PE transpose +
  GpSimd copy builds OT[(f,h), m]; one batched matmul per 128-k half
  against W[fh, k] accumulates over 64 h-chunks.  Epilogue: stable
  softplus (relu + ln(1+exp(-|x|)) on ScalarE) + per-partition bias,
  stored transposed [K, m] and untransposed on the host.

Queue discipline: sync/scalar HWDGE rings carry ONLY the big streaming
loads (et tiles, consts, w2); redistribute + output stores ride GpSimd
SWDGE so a semaphore-waiting small DMA never blocks the stream FIFO.
"""

import sys

import numpy as np

try:
    import concourse.bass as bass  # noqa: F401
except ImportError:
    sys.path.insert(0, "/opt/trn_rl_repo")

import concourse.bass as bass
import concourse.mybir as mybir
import concourse.tile as tile
from concourse import bacc
from concourse.bass_utils import run_bass_kernel_spmd
from concourse.masks import make_identity

F32 = mybir.dt.float32
F16 = mybir.dt.float16
NP_F16 = np.float16

N_CORES = 8
NA = 2048          # total atoms (n dimension)
F = 64             # dist_exp_size
H = 128            # atom_emb_size
K = 256            # hidden_size
M_SH = NA // N_CORES   # 256 rows per core
M_BLK = 128            # m-block (PSUM column count)
JJ = NA // 128         # n-chunks per row (16)
QN = M_BLK // 4        # q0 groups per block (32)


def build(m_sh=M_SH, na=NA, e_bufs=5):
    jj = na // 128
    n_mb = m_sh // M_BLK
    qn = QN
    kh_n = K // 128

    nc = bacc.Bacc(None, target_bir_lowering=False)
    # e_prep[p, mb, q0, r, j*64+f] = E[mb*128 + r*32 + q0, 16p+j, f]
    ep = nc.declare_dram_parameter(
        "e_prep", [128, n_mb, qn, 4, jj * F], F16, isOutput=False
    )
    # a_send[p, m*jj + j] = A[m, p*jj + j]
    a_send = nc.declare_dram_parameter("a_send", [128, m_sh * jj], F16, isOutput=False)
    # emb_prep[p, mb, h] = emb[mb*128 + p, h]
    embp = nc.declare_dram_parameter("emb_prep", [128, n_mb, H], F16, isOutput=False)
    # w2_prep[p, c, k] = W.reshape(F*H, K)[c*128 + p, k]   (c == f, p == h)
    w2p = nc.declare_dram_parameter("w2_prep", [128, F, K], F16, isOutput=False)
    bias = nc.declare_dram_parameter("bias", [128, kh_n], F32, isOutput=False)
    out = nc.declare_dram_parameter("out", [K, m_sh], F32, isOutput=True)

    AF = mybir.ActivationFunctionType

    with tile.TileContext(nc) as tc:
        with (
            tc.tile_pool(name="const", bufs=1) as cpool,
            tc.tile_pool(name="epool", bufs=e_bufs) as epool,
            tc.tile_pool(name="ot", bufs=2) as otpool,
            tc.tile_pool(name="tmp", bufs=3) as tmppool,
            tc.tile_pool(name="small", bufs=2) as smallpool,
            tc.tile_pool(name="outp", bufs=4) as outpool,
            tc.tile_pool(name="ps_adj", bufs=1, space="PSUM") as ps_adj_pool,
            tc.tile_pool(name="ps_t", bufs=2, space="PSUM") as ps_t_pool,
            tc.tile_pool(name="ps_f", bufs=2, space="PSUM") as ps_f_pool,
        ):
            # constants: a_send first (gates first matmul), all on sync ring
            a_sb = cpool.tile([128, m_sh * jj], F16)
            nc.sync.dma_start(a_sb[:], a_send[:])
            emb_sb = cpool.tile([128, n_mb, H], F16)
            nc.sync.dma_start(emb_sb[:], embp[:])
            bias_sb = cpool.tile([128, kh_n], F32)
            nc.sync.dma_start(bias_sb[:], bias[:])
            ident = cpool.tile([128, 128], F16)
            make_identity(nc, ident[:])
            w2_sb = cpool.tile([128, F, K], F16)

            for mb in range(n_mb):
                # ---- step 1: stream E, accumulate adj_exp in PSUM ----
                psum_adj = ps_adj_pool.tile([128, qn * F], F32)
                nc.vector.memset(psum_adj[:], 0.0)
                for q in range(qn // 2):  # q0-pairs, one 2MB DMA each
                    et = epool.tile([128, 2, 4, jj * F], F16)
                    eng = nc.sync if q % 2 == 0 else nc.scalar
                    eng.dma_start(et[:], ep[:, mb, 2 * q : 2 * q + 2])
                    for q01 in range(2):
                        q0 = 2 * q + q01
                        for j in range(jj):
                            for r in range(4):
                                m = mb * M_BLK + r * qn + q0
                                prow = 32 * r
                                nc.tensor.matmul(
                                    psum_adj[
                                        prow : prow + 1, q0 * F : (q0 + 1) * F
                                    ],
                                    lhsT=a_sb[:, m * jj + j : m * jj + j + 1],
                                    rhs=et[:, q01, r, j * F : (j + 1) * F],
                                    start=False,
                                    stop=(j == jj - 1),
                                    skip_group_check=True,
                                    tile_position=(0, prow),
                                )
                if mb == 0:
                    # w2 rides the scalar ring between block 0 and 1 et
                    # streams; lands well before step 2 needs it.
                    nc.scalar.dma_start(w2_sb[:], w2p[:])

                # ---- psum -> scratch (two engines, half each) ----
                scratch = smallpool.tile([128, qn * F], F32, tag="scr")
                half = qn * F // 2
                nc.vector.tensor_copy(scratch[:, :half], psum_adj[:, :half])
                nc.scalar.copy(scratch[:, half:], psum_adj[:, half:])

                # ---- redistribute to adj_exp[m_loc, f] via SWDGE ----
                adjexp_sb = smallpool.tile([128, F], F32, tag="adjexp")
                for r in range(4):
                    nc.gpsimd.dma_start(
                        adjexp_sb[r * qn : (r + 1) * qn, :],
                        scratch[32 * r : 32 * r + 1, :].rearrange(
                            "o (m f) -> o m f", f=F
                        ),
                    )

                # ---- OT[(f,h), m] build: DVE scalar-mul + PE transpose ----
                ot = otpool.tile([128, F, M_BLK], F16)
                for f in range(F):
                    tmp_o = tmppool.tile([128, H], F16)
                    nc.vector.tensor_scalar_mul(
                        tmp_o[:], emb_sb[:, mb, :], adjexp_sb[:, f : f + 1]
                    )
                    psum_o = ps_t_pool.tile([128, 128], F16, tag="tr")
                    nc.tensor.transpose(psum_o[:], tmp_o[:], ident[:])
                    nc.scalar.copy(ot[:, f, :], psum_o[:])

                # ---- step 2: feat_T[k, m] = sum_c w2_c.T @ OT_c ----
                for kh in range(kh_n):
                    psum_f = ps_f_pool.tile([128, M_BLK], F32)
                    for c in range(F):
                        nc.tensor.matmul(
                            psum_f[:],
                            lhsT=w2_sb[:, c, kh * 128 : (kh + 1) * 128],
                            rhs=ot[:, c, :],
                            start=(c == 0),
                            stop=(c == F - 1),
                        )
                    # stable softplus: relu(x) + ln(1 + exp(-min(|x|,30)))
                    ab = outpool.tile([128, M_BLK], F32, tag="sp_t")
                    nc.scalar.activation(ab[:], psum_f[:], AF.Abs)
                    nc.vector.tensor_scalar_min(ab[:], ab[:], 30.0)
                    nc.scalar.activation(ab[:], ab[:], AF.Exp, scale=-1.0)
                    nc.scalar.activation(ab[:], ab[:], AF.Ln, bias=1.0)
                    sp_sb = outpool.tile([128, M_BLK], F32)
                    nc.scalar.activation(sp_sb[:], psum_f[:], AF.Relu)
                    nc.vector.tensor_add(sp_sb[:], sp_sb[:], ab[:])
                    nc.vector.tensor_scalar_add(
                        sp_sb[:], sp_sb[:], bias_sb[:, kh : kh + 1]
                    )
                    nc.gpsimd.dma_start(
                        out[kh * 128 : (kh + 1) * 128, mb * M_BLK : (mb + 1) * M_BLK],
                        sp_sb[:],
                    )
    nc.compile()
    return nc


def prep_inputs(dist_adj, dist_exp, atom_emb, bilinear_w, bilinear_b, n_cores=N_CORES):
    """Shard + host-side layout prep. Returns in_maps for run_bass_kernel_spmd."""
    na = dist_adj.shape[1]
    m_sh = dist_adj.shape[0] // n_cores
    jj = na // 128
    n_mb = m_sh // M_BLK
    f, h, k = bilinear_w.shape
    # w2_prep[p=h, c=f, k]
    w2 = np.ascontiguousarray(
        np.asarray(bilinear_w).transpose(1, 0, 2)
    ).astype(NP_F16)
    bias = np.ascontiguousarray(
        np.asarray(bilinear_b, dtype=np.float32).reshape(k // 128, 128).T
    )
    de = np.asarray(dist_exp)
    in_maps = []
    for c in range(n_cores):
        sl = slice(c * m_sh, (c + 1) * m_sh)
        a = np.asarray(dist_adj[sl], dtype=np.float32)
        # a_send[p, m*jj + j] = A[m, p*jj + j]
        a_send = np.ascontiguousarray(
            a.reshape(m_sh, 128, jj).transpose(1, 0, 2).reshape(128, m_sh * jj)
        ).astype(NP_F16)
        # e_prep[p, mb, q0, r, j*64+f] = E[mb*128 + r*32 + q0, 16p+j, f]
        e_prep = (
            de[sl]
            .reshape(n_mb, 4, QN, 128, jj, f)
            .transpose(3, 0, 2, 1, 4, 5)
            .reshape(128, n_mb, QN, 4, jj * f)
            .astype(NP_F16)
        )
        emb_prep = np.ascontiguousarray(
            np.asarray(atom_emb[sl]).reshape(n_mb, 128, h).transpose(1, 0, 2)
        ).astype(NP_F16)
        in_maps.append(
            {
                "e_prep": np.ascontiguousarray(e_prep),
                "a_send": a_send,
                "emb_prep": emb_prep,
                "w2_prep": w2,
                "bias": bias,
            }
        )
    return in_maps


_NC_CACHE = {}


def _get_nc():
    if "nc" not in _NC_CACHE:
        _NC_CACHE["nc"] = build()
    return _NC_CACHE["nc"]


def assemble(results):
    """Gather per-core "out" tensors ([K, m_sh] each) into the full [N, K]."""
    return np.concatenate([r["out"].T for r in results], axis=0)


def kernel(dist_adj, dist_exp, atom_emb, bilinear_w, bilinear_b):
    nc = _get_nc()
    in_maps = prep_inputs(dist_adj, dist_exp, atom_emb, bilinear_w, bilinear_b)
    res = run_bass_kernel_spmd(nc, in_maps, core_ids=list(range(N_CORES)))
    return assemble(res.results)
